# revision 10
# baseline (speedup 1.0000x reference)
"""Converged-inhibition kernel for Trainium2 (8 NeuronCores, data-parallel).

The reference computes, per pixel (n, h, w):
    y = IFFT(FFT(x_c) / FFT(delta - filter)).real      (C = 63 channels)

Dividing by a fixed filter's DFT and inverse-transforming is a circular
deconvolution along the channel axis: y = G @ x with G the 63x63 circulant
matrix built from g = IFFT(1 / FFT(delta - filter)).real.  So the whole op
is one (63, 63) @ (63, N*H*W) matmul, embarrassingly parallel over pixels.

Device mapping: batch dim (64) sharded over 8 cores.  Since the contraction
dim (63) uses less than half the 128-wide PE array, two batches are stacked
per matmul column via a 126x126 block-diagonal weight, doubling PE
throughput.  The kernel is HBM-bandwidth bound (~50.6 MB per core).
"""

import os
import numpy as np

# Problem geometry (hardcoded: kernel.py must be self-contained).
C = 63
N_BATCH = 64
H = W = 112
HW = H * W                      # 12544
N_CORES = 8
B_PER_CORE = N_BATCH // N_CORES  # 8
P = 2 * C                       # 126 partitions = 2 batches stacked
ROWS = B_PER_CORE * C           # 504
N_GROUPS = B_PER_CORE // 2      # 4 batch-pairs per core
CHUNK = HW // int(os.environ.get("CI_NCHUNK", "4"))  # free-dim elements per pipelined chunk
N_CHUNKS = HW // CHUNK          # 4
MM_N = int(os.environ.get("CI_MM_N", "448"))  # moving free-dim per matmul (one PSUM bank; 7*448=3136)

# Matmul operand dtype.  "float16" halves the HBM traffic (the kernel's
# roofline) at ~4e-4 rel err, far under the 2e-2 gate; "float32r" is the
# full-precision fallback (~1.6e-4), "float32" the exact one.
MM_DTYPE = os.environ.get("CI_MM_DTYPE", "float16")

_PROG_CACHE = {}


def _build_program(mm_dtype_name):
    import concourse.bacc as bacc
    import concourse.mybir as mybir
    from concourse import tile

    # Bacc (not raw Bass): its compile() splits multi-semaphore waits into
    # event-semaphore chains (HW allows only one wait per instruction).
    nc = bacc.Bacc("TRN2", target_bir_lowering=False, debug=False)
    # For float32r (fp32 with 11-bit mantissa, full-rate PE path) the BIR
    # verifier requires every matmul operand's producer to emit float32r —
    # declaring the DRAM inputs and SBUF tiles as float32r makes the DMA that
    # producer; the host pre-rounds the arrays to the representable set.
    mm_dt = getattr(mybir.dt, mm_dtype_name)
    # 16-bit mode also writes the output as fp16 (host upcasts): HBM traffic
    # is the roofline, so halving the store stream matters as much as the load.
    out_dt = mybir.dt.float16 if mm_dtype_name == "float16" else mybir.dt.float32
    x_d = nc.dram_tensor("x", [ROWS, HW], mm_dt, kind="ExternalInput").ap()
    w_d = nc.dram_tensor("w", [P, P], mm_dt, kind="ExternalInput").ap()
    y_d = nc.dram_tensor("y", [ROWS, HW], out_dt, kind="ExternalOutput").ap()

    with tile.TileContext(nc) as tc:
        with (
            tc.tile_pool(name="wp", bufs=1) as wp,
            tc.tile_pool(name="xp", bufs=int(os.environ.get("CI_XBUFS", "6"))) as xp,
            tc.tile_pool(name="yp", bufs=int(os.environ.get("CI_YBUFS", "6"))) as yp,
            tc.tile_pool(name="pp", bufs=8, space="PSUM") as pp,
        ):
            w_t = wp.tile([P, P], mm_dt)

            # Chunk schedule per group: uniform CHUNK-sized pieces, except the
            # first group starts with two half-chunks so the first store is
            # issued ~6us earlier -- the HBM pipe only reaches full rate once
            # both the load and store streams are active.
            def group_sched(g):
                if g == 0 and os.environ.get("CI_HEADTAPER", "1") == "1":
                    cuts = [0, CHUNK // 2, CHUNK] + list(
                        range(2 * CHUNK, HW, CHUNK)
                    ) + [HW]
                else:
                    cuts = list(range(0, HW, CHUNK)) + [HW]
                return list(zip(cuts[:-1], cuts[1:]))

            first = True
            ci = 0  # global chunk index
            for g in range(N_GROUPS):
                r0 = g * P
                for c0, c1 in group_sched(g):
                    sz = c1 - c0
                    if first:
                        # w is tiny (32KB) but gates the first matmul: load it
                        # before the x stream so MMs start the moment x0 lands.
                        nc.sync.dma_start(out=w_t[:], in_=w_d[:])
                        first = False
                    xt = xp.tile([P, sz], mm_dt, tag="xt")
                    nc.sync.dma_start(out=xt[:], in_=x_d[r0 : r0 + P, c0:c1])
                    yt = yp.tile([P, sz], out_dt, tag="yt")
                    # The PSUM->SBUF casts are split between DVE and ACT: one
                    # engine alone (~100-150 G elem/s on fp32 PSUM reads) would
                    # become the bottleneck once the HBM streams shrink to fp16.
                    # Alternate which engine gets the extra odd tile per chunk.
                    for fi, f0 in enumerate(range(0, sz, MM_N)):
                        n = min(MM_N, sz - f0)
                        ps = pp.tile([P, MM_N], mybir.dt.float32, tag="ps")
                        nc.tensor.matmul(
                            ps[:, :n],
                            w_t[:],
                            xt[:, f0 : f0 + n],
                            start=True,
                            stop=True,
                        )
                        use_vec = (fi % 2 == 0) if (ci % 2 == 0) else (fi % 2 == 1)
                        if use_vec:
                            nc.vector.tensor_copy(yt[:, f0 : f0 + n], ps[:, :n])
                        else:
                            nc.scalar.copy(yt[:, f0 : f0 + n], ps[:, :n])
                    nc.scalar.dma_start(out=y_d[r0 : r0 + P, c0:c1], in_=yt[:])
                    ci += 1
    nc.compile()
    return nc


def _get_program():
    nc = _PROG_CACHE.get(MM_DTYPE)
    if nc is None:
        nc = _build_program(MM_DTYPE)
        _PROG_CACHE[MM_DTYPE] = nc
    return nc


def _weight_matrix(inhibition_filter, kronecker_delta):
    """126x126 block-diagonal lhsT = blockdiag(G.T, G.T), float32."""
    filt = np.asarray(inhibition_filter, dtype=np.float64).ravel()
    kd = np.asarray(kronecker_delta, dtype=np.float64).ravel()
    fk = np.fft.fft(kd - filt)
    g = np.real(np.fft.ifft(1.0 / fk))
    idx = (np.arange(C)[:, None] - np.arange(C)[None, :]) % C
    G = g[idx]  # G[c_out, c_in] = g[(c_out - c_in) mod C]
    lhsT = np.zeros((P, P), dtype=np.float32)
    GT = np.ascontiguousarray(G.T).astype(np.float32)  # lhsT[k, m] = G[m, k]
    lhsT[:C, :C] = GT
    lhsT[C:, C:] = GT
    return lhsT


def _round_fp32r(a):
    """Round fp32 to float32r's representable set (11-bit mantissa, RNE)."""
    b = a.view(np.uint32)
    lsb = (b >> 12) & 1
    out = ((b + 0x7FF + lsb) & 0xFFFFF000).astype(np.uint32)
    return out.view(np.float32)


LAST_RESULTS = None  # BassKernelResults of the most recent run (for profiling)


def kernel(activations, inhibition_filter, kronecker_delta):
    global LAST_RESULTS
    from concourse.bass_utils import run_bass_kernel_spmd

    acts = np.ascontiguousarray(np.asarray(activations, dtype=np.float32))
    assert acts.shape == (N_BATCH, C, H, W)
    w = _weight_matrix(inhibition_filter, kronecker_delta)
    if MM_DTYPE == "float32r":
        acts = _round_fp32r(acts)
        w = _round_fp32r(w)
    elif MM_DTYPE == "float16":
        acts = acts.astype(np.float16)
        w = w.astype(np.float16)

    nc = _get_program()
    in_maps = []
    for i in range(N_CORES):
        xs = acts[i * B_PER_CORE : (i + 1) * B_PER_CORE].reshape(ROWS, HW)
        in_maps.append({"x": np.ascontiguousarray(xs), "w": w})

    kw = {}
    tc_env = os.environ.get("CI_TRACE_CORES")
    if tc_env:
        kw["trace_cores"] = [int(c) for c in tc_env.split(",")]
    try:
        res = run_bass_kernel_spmd(nc, in_maps, list(range(N_CORES)), **kw)
    except Exception:
        # A previously wedged device can fail the first execute; one retry
        # after requesting a core reset usually clears it.
        os.environ.setdefault("NEURON_RT_RESET_CORES", "1")
        res = run_bass_kernel_spmd(nc, in_maps, list(range(N_CORES)), **kw)
    LAST_RESULTS = res

    out = np.concatenate(
        [res.results[i]["y"].reshape(B_PER_CORE, C, H, W) for i in range(N_CORES)],
        axis=0,
    )
    return out.astype(np.float32, copy=False)

